# revision 4
# baseline (speedup 1.0000x reference)
"""Multi-head attention (B=4, S=2048, D=1024, H=16, causal, all-valid padding)
for 8 Trainium2 NeuronCores.

Sharding: hybrid data-parallel x tensor-parallel. Core c handles batch
b = c // 2 and head-group g = c % 2 (8 heads, 512 channels each). Each core
computes its head-group's Q/K/V projections, causal attention, and the
partial output projection through its slice of Wo. The host sums the two
head-group partials per batch (the row-parallel all-reduce) and stacks
batches.

On-chip layout (per core):
  - x fed pre-transposed (D, S) so D lands on partitions for the QKV matmuls.
  - Q^T, K^T kept as [128ch, S] tiles (two 64-ch heads stacked per pair) so
    scores are computed transposed: S^T[k,q] = K_tile @ Q^T, with the two
    heads of a pair row-packed into the PE array (dk=64 each).
  - P^T = exp(S^T/8) via ACT straight out of PSUM, causal-masked by a
    precomputed bf16 mask multiply on DVE (only on diagonal tiles; fully
    masked tiles are skipped).
  - ctx^T accumulates in PSUM via col-packed V-matmuls; softmax denominators
    accumulate in a parallel PSUM bank via an all-ones stationary operand
    (M=64 -> denominator pre-broadcast across partitions), so normalization
    is one reciprocal + one multiply per (pair, q-chunk).
  - y = ctx_norm^T.T @ Wo^T slices, accumulated over the 4 channel tiles.
"""

import numpy as np
import ml_dtypes

B, S, D, H = 4, 2048, 1024, 16
DK = D // H            # 64
CH = D // 2            # 512 local channels per core (8 heads)
NPAIR = 4              # pairs of heads per core (2 heads x 64ch = 128ch tile)
SCHUNK = 512           # s-chunk (q-chunk) width
KTILE = 128            # k-tile width
NDT = D // 128         # 8 d-tiles (contraction for projections)

_BF16 = ml_dtypes.bfloat16


def _build_nc(s_len):
    import concourse.bass as bass
    import concourse.mybir as mybir
    import concourse.tile as tile
    from concourse import bacc

    f32 = mybir.dt.float32
    bf16 = mybir.dt.bfloat16
    Exp = mybir.ActivationFunctionType.Exp

    nsc = s_len // SCHUNK          # s-chunks / q-chunks
    nkt_total = s_len // KTILE     # k-tiles
    nqt = s_len // 128             # q row tiles for Wo

    nc = bacc.Bacc("TRN2", target_bir_lowering=False, debug=False)

    xq_d = nc.dram_tensor("xqT", [D, s_len], bf16, kind="ExternalInput")
    xk_d = nc.dram_tensor("xkT", [D, s_len], bf16, kind="ExternalInput")
    xv_d = nc.dram_tensor("xvT", [D, s_len], bf16, kind="ExternalInput")
    wq_d = nc.dram_tensor("wqT", [D, CH], bf16, kind="ExternalInput")
    wk_d = nc.dram_tensor("wkT", [D, CH], bf16, kind="ExternalInput")
    wv_d = nc.dram_tensor("wvT", [D, CH], bf16, kind="ExternalInput")
    wo_d = nc.dram_tensor("woT", [CH, D], bf16, kind="ExternalInput")
    mask_d = nc.dram_tensor("masks", [4, 128, SCHUNK], bf16, kind="ExternalInput")
    y_d = nc.dram_tensor("y", [s_len, D], f32, kind="ExternalOutput")

    xq_r = xq_d[:, :].rearrange("(d p) s -> p d s", p=128)
    xk_r = xk_d[:, :].rearrange("(d p) s -> p d s", p=128)
    xv_r = xv_d[:, :].rearrange("(d p) s -> p d s", p=128)

    with tile.TileContext(nc) as tc:
        from contextlib import ExitStack

        with ExitStack() as ctx:
            const_pool = ctx.enter_context(tc.tile_pool(name="const", bufs=1))
            w_pool = ctx.enter_context(tc.tile_pool(name="weights", bufs=1))
            qt_pool = ctx.enter_context(tc.tile_pool(name="qt", bufs=NPAIR * nsc))
            kt_pool = ctx.enter_context(tc.tile_pool(name="kt", bufs=NPAIR * nsc))
            v_pool = ctx.enter_context(tc.tile_pool(name="v", bufs=nkt_total))
            ctx_pool = ctx.enter_context(tc.tile_pool(name="ctx", bufs=NPAIR * nsc))
            x_pool = ctx.enter_context(tc.tile_pool(name="x", bufs=4))
            pt_pool = ctx.enter_context(tc.tile_pool(name="pt", bufs=4))
            ev_pool = ctx.enter_context(tc.tile_pool(name="ev", bufs=2))
            y_pool = ctx.enter_context(tc.tile_pool(name="yout", bufs=3))
            qkv_ps = ctx.enter_context(
                tc.tile_pool(name="qkv_ps", bufs=2, space="PSUM"))
            st_ps = ctx.enter_context(
                tc.tile_pool(name="st_ps", bufs=2, space="PSUM"))
            ctx_ps_pool = ctx.enter_context(
                tc.tile_pool(name="ctx_ps", bufs=1, space="PSUM"))
            den_ps_pool = ctx.enter_context(
                tc.tile_pool(name="den_ps", bufs=1, space="PSUM"))

            ones_sb = const_pool.tile([128, 64], bf16)
            nc.any.memset(ones_sb[:, :], 1.0)
            mask_sb = const_pool.tile([128, 4, SCHUNK], bf16)
            nc.sync.dma_start(
                mask_sb[:, :, :], mask_d[:, :, :].rearrange("r p m -> p r m"))

            wq_sb = w_pool.tile([128, NDT, CH], bf16)
            nc.sync.dma_start(
                wq_sb[:, :, :], wq_d[:, :].rearrange("(d p) c -> p d c", p=128))
            wk_sb = w_pool.tile([128, NDT, CH], bf16)
            nc.sync.dma_start(
                wk_sb[:, :, :], wk_d[:, :].rearrange("(d p) c -> p d c", p=128))
            wv_sb = w_pool.tile([128, NDT, CH], bf16)
            nc.sync.dma_start(
                wv_sb[:, :, :], wv_d[:, :].rearrange("(d p) c -> p d c", p=128))
            wo_sb = w_pool.tile([128, NPAIR, D], bf16)
            nc.sync.dma_start(
                wo_sb[:, :, :], wo_d[:, :].rearrange("(c p) o -> p c o", p=128))

            qt_tiles = {}
            kt_tiles = {}
            v_tiles = {}
            ctx_tiles = {}

            for sc in range(nsc):
                # ---- projections for s-chunk sc ----
                xq_t = x_pool.tile([128, NDT, SCHUNK], bf16, tag="x")
                nc.sync.dma_start(
                    xq_t[:, :, :], xq_r[:, :, sc * SCHUNK:(sc + 1) * SCHUNK])
                xk_t = x_pool.tile([128, NDT, SCHUNK], bf16, tag="x")
                nc.sync.dma_start(
                    xk_t[:, :, :], xk_r[:, :, sc * SCHUNK:(sc + 1) * SCHUNK])
                xv_t = x_pool.tile([128, NDT, SCHUNK], bf16, tag="x")
                nc.sync.dma_start(
                    xv_t[:, :, :], xv_r[:, :, sc * SCHUNK:(sc + 1) * SCHUNK])

                for m in range(NPAIR):
                    ps = qkv_ps.tile([128, SCHUNK], f32, tag="qkv")
                    for d in range(NDT):
                        nc.tensor.matmul(
                            ps[:, :],
                            lhsT=wq_sb[:, d, m * 128:(m + 1) * 128],
                            rhs=xq_t[:, d, :],
                            start=(d == 0), stop=(d == NDT - 1))
                    t = qt_pool.tile([128, SCHUNK], bf16, tag="qt",
                                     name=f"qt_{m}_{sc}")
                    nc.vector.tensor_copy(t[:, :], ps[:, :])
                    qt_tiles[(m, sc)] = t
                for m in range(NPAIR):
                    ps = qkv_ps.tile([128, SCHUNK], f32, tag="qkv")
                    for d in range(NDT):
                        nc.tensor.matmul(
                            ps[:, :],
                            lhsT=wk_sb[:, d, m * 128:(m + 1) * 128],
                            rhs=xk_t[:, d, :],
                            start=(d == 0), stop=(d == NDT - 1))
                    t = kt_pool.tile([128, SCHUNK], bf16, tag="kt",
                                     name=f"kt_{m}_{sc}")
                    nc.vector.tensor_copy(t[:, :], ps[:, :])
                    kt_tiles[(m, sc)] = t
                for ss in range(SCHUNK // 128):
                    ps = qkv_ps.tile([128, CH], f32, tag="qkv")
                    for d in range(NDT):
                        nc.tensor.matmul(
                            ps[:, :],
                            lhsT=xv_t[:, d, ss * 128:(ss + 1) * 128],
                            rhs=wv_sb[:, d, :],
                            start=(d == 0), stop=(d == NDT - 1))
                    kt_idx = sc * (SCHUNK // 128) + ss
                    t = v_pool.tile([128, CH], bf16, tag="v",
                                    name=f"v_{kt_idx}")
                    nc.vector.tensor_copy(t[:, :], ps[:, :])
                    v_tiles[kt_idx] = t

                # ---- attention for q-chunk qc = sc ----
                qc = sc
                nkt = (qc + 1) * (SCHUNK // KTILE)  # causal: k-tiles 0..nkt-1
                for pair in range(NPAIR):
                    ctx_p = ctx_ps_pool.tile([128, SCHUNK], f32, tag="ctxps")
                    den_p = den_ps_pool.tile([128, SCHUNK], f32, tag="denps")

                    def emit_scores(kt):
                        st = st_ps.tile([128, 2 * SCHUNK], f32, tag="st")
                        ktile = kt_tiles[(pair, kt // 4)]
                        qtile = qt_tiles[(pair, qc)]
                        for h in range(2):
                            nc.tensor.matmul(
                                st[:, h * SCHUNK:(h + 1) * SCHUNK],
                                lhsT=ktile[h * 64:(h + 1) * 64,
                                           (kt % 4) * KTILE:(kt % 4 + 1) * KTILE],
                                rhs=qtile[h * 64:(h + 1) * 64, :],
                                start=True, stop=True)
                        pt = pt_pool.tile([128, 2 * SCHUNK], bf16, tag="pt")
                        nc.scalar.activation(pt[:, :], st[:, :], Exp, scale=0.125)
                        r = kt - qc * (SCHUNK // KTILE)
                        if r >= 0:  # diagonal tile: apply causal mask
                            for h in range(2):
                                nc.vector.tensor_mul(
                                    pt[:, h * SCHUNK:(h + 1) * SCHUNK],
                                    pt[:, h * SCHUNK:(h + 1) * SCHUNK],
                                    mask_sb[:, r, :])
                        return pt

                    pt_cur = emit_scores(0)
                    for kt in range(nkt):
                        pt_next = emit_scores(kt + 1) if kt + 1 < nkt else None
                        vt = v_tiles[kt]
                        for h in range(2):
                            hl = pair * 2 + h
                            nc.tensor.matmul(
                                ctx_p[h * 64:(h + 1) * 64, :],
                                lhsT=vt[:, hl * 64:(hl + 1) * 64],
                                rhs=pt_cur[:, h * SCHUNK:(h + 1) * SCHUNK],
                                start=(kt == 0), stop=(kt == nkt - 1),
                                tile_position=(0, h * 64),
                                skip_group_check=True)
                            nc.tensor.matmul(
                                den_p[h * 64:(h + 1) * 64, :],
                                lhsT=ones_sb[:, :],
                                rhs=pt_cur[:, h * SCHUNK:(h + 1) * SCHUNK],
                                start=(kt == 0), stop=(kt == nkt - 1),
                                tile_position=(0, h * 64),
                                skip_group_check=True)
                        pt_cur = pt_next

                    rec = ev_pool.tile([128, SCHUNK], f32, tag="rec")
                    nc.vector.reciprocal(rec[:, :], den_p[:, :])
                    t = ctx_pool.tile([128, SCHUNK], bf16, tag="ctx",
                                      name=f"ctx_{pair}_{qc}")
                    nc.vector.tensor_mul(t[:, :], ctx_p[:, :], rec[:, :])
                    ctx_tiles[(pair, qc)] = t

            # ---- output projection ----
            for qt in range(nqt):
                qc = qt // 4
                for oc in range(D // 512):
                    ps = qkv_ps.tile([128, 512], f32, tag="qkv")
                    for cj in range(NPAIR):
                        nc.tensor.matmul(
                            ps[:, :],
                            lhsT=ctx_tiles[(cj, qc)][:, (qt % 4) * 128:
                                                     (qt % 4 + 1) * 128],
                            rhs=wo_sb[:, cj, oc * 512:(oc + 1) * 512],
                            start=(cj == 0), stop=(cj == NPAIR - 1))
                    yt = y_pool.tile([128, 512], f32, tag="yout")
                    nc.vector.tensor_copy(yt[:, :], ps[:, :])
                    nc.sync.dma_start(
                        y_d[qt * 128:(qt + 1) * 128, oc * 512:(oc + 1) * 512],
                        yt[:, :])

    nc.finalize()
    return nc


def _make_masks():
    ki = np.arange(128)[:, None]
    qi = np.arange(SCHUNK)[None, :]
    m = np.stack([(qi >= ki + 128 * r) for r in range(4)]).astype(_BF16)
    return m


def _host_shards(x_query, x_key, x_value, Wq, Wk, Wv, Wo, s_len):
    """Per-core input dicts. Core c: batch c//2, head-group c%2."""
    masks = _make_masks()
    in_maps = []
    for c in range(8):
        b, g = c // 2, c % 2
        lo, hi = g * CH, (g + 1) * CH
        in_maps.append({
            "xqT": np.ascontiguousarray(x_query[b, :s_len].T).astype(_BF16),
            "xkT": np.ascontiguousarray(x_key[b, :s_len].T).astype(_BF16),
            "xvT": np.ascontiguousarray(x_value[b, :s_len].T).astype(_BF16),
            "wqT": np.ascontiguousarray(Wq[lo:hi, :].T).astype(_BF16),
            "wkT": np.ascontiguousarray(Wk[lo:hi, :].T).astype(_BF16),
            "wvT": np.ascontiguousarray(Wv[lo:hi, :].T).astype(_BF16),
            "woT": np.ascontiguousarray(Wo[:, lo:hi].T).astype(_BF16),
            "masks": masks,
        })
    return in_maps


_NC_CACHE = {}


def _get_nc(s_len):
    if s_len not in _NC_CACHE:
        _NC_CACHE[s_len] = _build_nc(s_len)
    return _NC_CACHE[s_len]


def kernel(x_query, x_key, x_value, attention_mask, Wq, Wk, Wv, Wo,
           _trace=False):
    from concourse.bass_utils import run_bass_kernel_spmd

    nc = _get_nc(S)
    in_maps = _host_shards(x_query, x_key, x_value, Wq, Wk, Wv, Wo, S)
    res = run_bass_kernel_spmd(nc, in_maps, core_ids=list(range(8)),
                               trace=_trace)
    y = np.empty((B, S, D), dtype=np.float32)
    for b in range(B):
        y[b] = res.results[2 * b]["y"].astype(np.float32) + \
            res.results[2 * b + 1]["y"].astype(np.float32)
    if _trace:
        return y, res
    return y
